# revision 8
# baseline (speedup 1.0000x reference)
"""Trainium2 Bass kernel for the 4-layer LSTM (T=128, B=64, H=1024).

Strategy: 4-stage layer pipeline x 2-way batch data-parallel = 8 cores.
Rank r: stage j = r % 4 (layer j), batch half = r // 4 (B_LOC = 32).
The two batch halves run identical, fully independent pipelines; the
c-chunk transport between consecutive stages is a 4-rank AllGather per
tick within each half's replica group ([[0,1,2,3],[4,5,6,7]]).

Everything lives in "transposed space" (zT = [4H, B_LOC]) so no per-step
transposes are needed:

  - Stage j per tick: (a) batched x@W for a G-step chunk of the previous
    layer's c outputs (gathered via AllGather), accumulated into PSUM;
    (b) G sequential LSTM steps, each accumulating h@U into the same PSUM
    (start=False) and applying gates; (c) write the chunk's cT (bf16) to a
    DRAM bounce buffer.
  - One 4-rank AllGather per tick moves c-chunks between stage cores; it
    overlaps the next tick's compute (chunks are consumed two ticks after
    production).
  - PSUM is double-buffered (4 banks per tick chunk), so tick tau+1's
    x@W matmuls can run while tick tau's gate elementwise chain drains.

Output = cell state of layer 3 at t=T-1 (rank 3 holds batch 0:32,
rank 7 holds batch 32:64).
"""

import sys

for p in ("/opt/trn_rl_repo",):
    if p not in sys.path:
        sys.path.insert(0, p)

import numpy as np
import ml_dtypes

T, B, H, L = 128, 64, 1024, 4
FH = 4 * H
KT = H // 128           # 8 K-tiles
MT = FH // 128          # 32 M-tiles
B_LOC = B // 2          # batch per core (2-way data parallel)
G = 2                   # steps per chunk
NCH = T // G            # chunks per layer
NTICKS = NCH + 2 * (L - 1) + 2   # schedule: stage j runs chunk k at tick k+2j+2
N_CORES = 8
GB = G * B_LOC          # chunk free-dim (steps x local batch)
MMPB = 512 // GB        # mm blocks per 2KB PSUM bank

_CACHE = {}


def _build(nticks=NTICKS):
    import concourse.bacc as bacc
    import concourse.bass as bass
    import concourse.mybir as mybir
    import concourse.tile as tile

    bf16, f32, i32 = mybir.dt.bfloat16, mybir.dt.float32, mybir.dt.int32
    AF = mybir.ActivationFunctionType
    Alu = mybir.AluOpType

    nc = bacc.Bacc("TRN2", target_bir_lowering=False, debug=False,
                   num_devices=N_CORES)

    w_in = nc.dram_tensor("w_loc", [H, FH], bf16, kind="ExternalInput")
    u_in = nc.dram_tensor("u_loc", [H, FH], bf16, kind="ExternalInput")
    src_static = nc.dram_tensor("src_static", [H, T * B_LOC], bf16,
                                kind="ExternalInput")
    rparam = nc.dram_tensor("rparam", [1, 2], i32, kind="ExternalInput")
    out_ext = nc.dram_tensor("out", [128, KT * B_LOC], f32,
                             kind="ExternalOutput")

    # DRAM bounce buffers (double-buffered by tick parity). The gather
    # output is addr_space=Shared so the 8-rank AllGather can take the
    # direct shared-output path (4-rank groups only support the slow
    # Mesh algorithm, ~34us per 128KB gather).
    c_out = [nc.dram_tensor(f"c_out{i}", [H, GB], bf16) for i in range(2)]
    gath = [nc.dram_tensor(f"gath{i}", [8, H, GB], bf16,
                           addr_space="Shared") for i in range(2)]

    with tile.TileContext(nc) as tc:
        with (
            tc.tile_pool(name="wp", bufs=1) as wp,
            tc.tile_pool(name="sp", bufs=1) as sp,
            tc.tile_pool(name="srcp", bufs=2) as srcp,
            tc.tile_pool(name="ewp", bufs=2) as ewp,
            tc.tile_pool(name="zp", bufs=2, space="PSUM") as zp,
        ):
            # ---- preamble -------------------------------------------------
            w_sb = wp.tile([128, KT * FH], bf16)   # W K-tile k at k*FH
            u_sb = wp.tile([128, KT * FH], bf16)
            for k in range(KT):
                nc.sync.dma_start(w_sb[:, k * FH:(k + 1) * FH],
                                  w_in[k * 128:(k + 1) * 128, :])
                nc.sync.dma_start(u_sb[:, k * FH:(k + 1) * FH],
                                  u_in[k * 128:(k + 1) * 128, :])

            rp_sb = sp.tile([1, 2], i32)
            nc.sync.dma_start(rp_sb[:], rparam[:])
            rv = nc.values_load(rp_sb[:1, 0:1].to_broadcast((1, 1)))
            rk = nc.values_load(rp_sb[:1, 1:2].to_broadcast((1, 1)))

            zsb = sp.tile([128, KT * GB], bf16)
            nc.gpsimd.memset(zsb[:], 0.0)
            for i in range(2):
                nc.sync.dma_start(
                    c_out[i].rearrange("(k p) n -> p k n", p=128),
                    zsb[:].rearrange("p (k n) -> p k n", k=KT))
                for s in range(8):
                    nc.sync.dma_start(
                        gath[i][s].rearrange("(k p) n -> p k n", p=128),
                        zsb[:].rearrange("p (k n) -> p k n", k=KT))

            # state (double-buffered by global step parity)
            cT = [sp.tile([128, KT * B_LOC], f32, name=f"cT{i}")
                  for i in range(2)]
            hT = [sp.tile([128, KT * B_LOC], bf16, name=f"hT{i}")
                  for i in range(2)]
            for i in range(2):
                nc.gpsimd.memset(cT[i][:], 0.0)
                nc.gpsimd.memset(hT[i][:], 0.0)

            gstep = 0  # global step counter for state parity

            # ---- tick loop ------------------------------------------------
            for tau in range(nticks):
                nc.gpsimd.collective_compute(
                    "AllGather", Alu.bypass,
                    replica_groups=[[0, 1, 2, 3, 4, 5, 6, 7]],
                    ins=[c_out[(tau - 1) % 2].ap().opt()],
                    outs=[gath[tau % 2].ap().opt()],
                )

                # state reset at each stage's first real tick
                if tau in (2, 4, 6, 8):
                    j = (tau - 2) // 2
                    with tc.If(rv == j):
                        nc.gpsimd.memset(cT[gstep % 2][:], 0.0)
                        nc.gpsimd.memset(hT[gstep % 2][:], 0.0)

                # source chunk for this tick: stage 0 from src_static,
                # stages>=1 from gathered slot (stage-1); slot via per-rank If.
                src_sb = srcp.tile([128, KT * GB], bf16, tag="src",
                                   name=f"src_{tau}")
                kchunk = min(max(tau - 2, 0), NCH - 1)
                with tc.If(rv == 0) as cmp:
                    nc.sync.dma_start(
                        src_sb[:].rearrange("p (k n) -> p k n", k=KT),
                        src_static[:, kchunk * GB:(kchunk + 1) * GB]
                        .rearrange("(k p) n -> p k n", p=128))
                with cmp.Else():
                    for r in (1, 2, 3, 5, 6, 7):
                        with tc.If(rk == r):
                            nc.sync.dma_start(
                                src_sb[:].rearrange("p (k n) -> p k n", k=KT),
                                gath[(tau - 1) % 2][r - 1]
                                .rearrange("(k p) n -> p k n", p=128))

                # chunk PSUM: [128, MT * GB] f32 = 4 banks at G=2, B_LOC=32
                psz = zp.tile([128, MT * GB], f32, tag="Z", name=f"psz_{tau}")
                # batched x@W for the chunk. PSUM start/stop are
                # bank-granular: only the first matmul touching a bank may
                # carry start=True (it clears the whole bank's has_written).
                for mm in range(MT):
                    for k in range(KT):
                        nc.tensor.matmul(
                            psz[:, mm * GB:(mm + 1) * GB],
                            w_sb[:, k * FH + mm * 128:k * FH + (mm + 1) * 128],
                            src_sb[:, k * GB:(k + 1) * GB],
                            start=(mm % MMPB == 0 and k == 0), stop=False,
                            skip_group_check=True,
                        )

                cbf = ewp.tile([128, KT * B_LOC], bf16, tag="cbf",
                               name=f"cbf_{tau}")
                for s in range(G):
                    h_prev = hT[gstep % 2]
                    c_prev = cT[gstep % 2]
                    h_new = hT[(gstep + 1) % 2]
                    c_new = cT[(gstep + 1) % 2]
                    # h @ U accumulated on top of x@W (+start=False)
                    for mm in range(MT):
                        for k in range(KT):
                            nc.tensor.matmul(
                                psz[:, mm * GB + s * B_LOC:
                                    mm * GB + (s + 1) * B_LOC],
                                u_sb[:, k * FH + mm * 128:
                                     k * FH + (mm + 1) * 128],
                                h_prev[:, k * B_LOC:(k + 1) * B_LOC],
                                start=False,
                                stop=(s == G - 1 and mm % MMPB == MMPB - 1
                                      and k == KT - 1),
                                skip_group_check=True,
                            )
                    # gates: mm 0-7 = i, 8-15 = f, 16-23 = g, 24-31 = o
                    # step-s columns: strided views [mm, s*B_LOC:(s+1)*B_LOC]
                    def zview(g0, g1, s=s):
                        return psz[:].rearrange(
                            "p (mm n) -> p mm n", n=GB
                        )[:, g0 * 8:g1 * 8, s * B_LOC:(s + 1) * B_LOC]
                    sif = ewp.tile([128, 2 * KT * B_LOC], f32, tag="sif",
                                   name=f"sif_{tau}_{s}")
                    tg = ewp.tile([128, KT * B_LOC], f32, tag="tg",
                                  name=f"tg_{tau}_{s}")
                    so = ewp.tile([128, KT * B_LOC], f32, tag="so",
                                  name=f"so_{tau}_{s}")
                    nc.scalar.activation(
                        sif[:].rearrange("p (mm n) -> p mm n", n=B_LOC),
                        zview(0, 2), AF.Sigmoid)
                    nc.scalar.activation(
                        tg[:].rearrange("p (mm n) -> p mm n", n=B_LOC),
                        zview(2, 3), AF.Tanh)
                    nc.scalar.activation(
                        so[:].rearrange("p (mm n) -> p mm n", n=B_LOC),
                        zview(3, 4), AF.Sigmoid)
                    fc = ewp.tile([128, KT * B_LOC], f32, tag="fc",
                                  name=f"fc_{tau}_{s}")
                    ig = ewp.tile([128, KT * B_LOC], f32, tag="ig",
                                  name=f"ig_{tau}_{s}")
                    nc.vector.tensor_tensor(fc[:], sif[:, KT * B_LOC:],
                                            c_prev[:], Alu.mult)
                    nc.vector.tensor_tensor(ig[:], sif[:, 0:KT * B_LOC],
                                            tg[:], Alu.mult)
                    nc.vector.tensor_tensor(c_new[:], fc[:], ig[:], Alu.add)
                    th = ewp.tile([128, KT * B_LOC], f32, tag="th",
                                  name=f"th_{tau}_{s}")
                    nc.scalar.activation(th[:], c_new[:], AF.Tanh)
                    nc.vector.tensor_tensor(h_new[:], so[:], th[:], Alu.mult)
                    # cast c for transport
                    nc.vector.tensor_copy(cbf[:], c_new[:])
                    nc.sync.dma_start(
                        c_out[tau % 2][:, s * B_LOC:(s + 1) * B_LOC]
                        .rearrange("(k p) n -> p k n", p=128),
                        cbf[:].rearrange("p (k n) -> p k n", k=KT))
                    gstep += 1

            # final state out (ranks 3 and 7 hold the answer)
            nc.sync.dma_start(out_ext[:], cT[gstep % 2][:])
    nc.finalize()
    return nc


def _prep_in_maps(inputs, W, U, b):
    xT = np.ascontiguousarray(
        inputs.astype(np.float32).transpose(2, 0, 1)  # [H, T, B]
    )
    halves = [
        np.ascontiguousarray(xT[:, :, :B_LOC].reshape(H, T * B_LOC))
        .astype(ml_dtypes.bfloat16),
        np.ascontiguousarray(xT[:, :, B_LOC:].reshape(H, T * B_LOC))
        .astype(ml_dtypes.bfloat16),
    ]
    zeros_src = np.zeros((H, T * B_LOC), dtype=ml_dtypes.bfloat16)
    Wb = W.astype(ml_dtypes.bfloat16)
    Ub = U.astype(ml_dtypes.bfloat16)
    in_maps = []
    for r in range(N_CORES):
        j = r % 4
        in_maps.append({
            "w_loc": np.ascontiguousarray(Wb[j]),
            "u_loc": np.ascontiguousarray(Ub[j]),
            "src_static": halves[r // 4] if j == 0 else zeros_src,
            "rparam": np.array([[j, r]], dtype=np.int32),
        })
    return in_maps


def kernel(inputs, W, U, b):
    assert not np.any(b), "nonzero bias not implemented"
    from concourse.bass_utils import run_bass_kernel_spmd

    if "nc" not in _CACHE:
        _CACHE["nc"] = _build()
    nc = _CACHE["nc"]
    in_maps = _prep_in_maps(inputs, W, U, b)
    res = run_bass_kernel_spmd(nc, in_maps, core_ids=list(range(N_CORES)))
    c = np.zeros((B, H), dtype=np.float32)
    for half, rank in ((0, 3), (1, 7)):
        ct = res.results[rank]["out"]  # [128, KT*B_LOC], k-tile k at k*B_LOC
        for k in range(KT):
            c[half * B_LOC:(half + 1) * B_LOC, k * 128:(k + 1) * 128] = \
                ct[:, k * B_LOC:(k + 1) * B_LOC].T
    return c
